# revision 1
# baseline (speedup 1.0000x reference)
"""Trainium2 Bass kernel for classical self-attention (B=1, N=4096, D=768, H=12, Hd=64).

Sharding across 8 NeuronCores (zero-collective SPMD):
  24 units = (head h in 0..11, row-half r in {0,1}); core c owns units
  [3c, 3c+2], reordered per core as [U0, U1, U2] with KV head-slots
  (0, 1, 0) so the program is identical on every core:
    U0 = (m2_head, solo_half), U1 = (solo_head, solo_half), U2 = (m2_head, 1-solo_half)
  where m2_head is the head appearing twice among the core's units.

Per core (all matmuls in float32r; out = lhsT.T @ rhs):
  - K^T/V^T/Q^T projections from a row-permuted x^T (key order permuted
    identically for K and V, so softmax/PV are unaffected).
  - scores^T tiles [128 keys, 512 qrows] -> exp on ACT (scale=1/8 folded in)
    -> PV with a ones-column appended to V so the softmax denominator
    accumulates for free in row 64 of the O^T PSUM tile.
  - out_proj partial = O^T.T @ w_out_cols^T, normalized by 1/denominator
    per query row on the way out of PSUM.
Host sums the 24 partial [2048, 768] blocks (12 heads per row-half) and
adds the output bias.
"""
import numpy as np
from functools import partial

H, Hd, N, D = 12, 64, 4096, 768
NC = 8
NKT = N // 128        # 32 key tiles
NQC = 2048 // 512     # 4 q-chunks per unit
KTG = 3               # key tiles per exp group (3 PSUM banks)


def _core_units(c):
    us = [(u // 2, u % 2) for u in range(3 * c, 3 * c + 3)]
    heads = [h for h, _ in us]
    m2 = max(set(heads), key=heads.count)
    solo_head, solo_half = next((h, r) for h, r in us if h != m2)
    return [(m2, solo_half), (solo_head, solo_half), (m2, 1 - solo_half)]


def _prep_core_inputs(c, x, w_qkv, w_out):
    U = _core_units(c)
    solo_half = U[0][1]
    slot_heads = [U[0][0], U[1][0]]

    xT = x.T  # [768, 4096]
    xT_r = np.ascontiguousarray(np.concatenate(
        [xT[:, 2048 * solo_half:2048 * (solo_half + 1)],
         xT[:, 2048 * (1 - solo_half):2048 * (2 - solo_half)]], axis=1))

    wk = np.stack([w_qkv[768 + h * 64: 768 + (h + 1) * 64] for h in slot_heads])
    wv = np.stack([w_qkv[1536 + h * 64: 1536 + (h + 1) * 64] for h in slot_heads])
    wq = np.stack([w_qkv[h * 64:(h + 1) * 64] for h, _ in U])
    # SBUF layouts: w*_l[p, t, m] = w*T[t*128+p, m] so device DMAs are contiguous.
    wk_l = np.ascontiguousarray(wk.reshape(128, 768).T.reshape(6, 128, 128).transpose(1, 0, 2))
    wv_l = np.ascontiguousarray(wv.reshape(128, 768).T.reshape(6, 128, 128).transpose(1, 0, 2))
    wq_l = np.ascontiguousarray(wq.reshape(192, 768).T.reshape(6, 128, 192).transpose(1, 0, 2))
    wo_l = np.ascontiguousarray(
        np.stack([w_out[:, h * 64:(h + 1) * 64].T for h, _ in U]).transpose(1, 0, 2))
    return dict(xT_r=xT_r, wk_l=wk_l, wv_l=wv_l, wq_l=wq_l, wo_l=wo_l,
                ident=np.eye(128, dtype=np.float32),
                ones_col=np.ones((128, 64), np.float32))


def _build_bass():
    import concourse.mybir as mybir
    import concourse.tile as tile
    from concourse import bacc

    f32 = mybir.dt.float32
    f32r = mybir.dt.float32r
    nc = bacc.Bacc(None, target_bir_lowering=False)

    xT_r = nc.dram_tensor("xT_r", [D, N], f32r, kind="ExternalInput")
    wk_l = nc.dram_tensor("wk_l", [128, 6, 128], f32r, kind="ExternalInput")
    wv_l = nc.dram_tensor("wv_l", [128, 6, 128], f32r, kind="ExternalInput")
    wq_l = nc.dram_tensor("wq_l", [128, 6, 192], f32r, kind="ExternalInput")
    wo_l = nc.dram_tensor("wo_l", [64, 3, D], f32r, kind="ExternalInput")
    ident_d = nc.dram_tensor("ident", [128, 128], f32r, kind="ExternalInput")
    ones_d = nc.dram_tensor("ones_col", [128, 64], f32r, kind="ExternalInput")
    out_part = nc.dram_tensor("out_part", [2, 2048, D], f32, kind="ExternalOutput")

    def r(ap):
        return ap

    with tile.TileContext(nc) as tc:
        with (
            tc.tile_pool(name="wpool", bufs=1) as wpool,
            tc.tile_pool(name="big", bufs=1) as big,
            tc.tile_pool(name="expp", bufs=3) as expp,
            tc.tile_pool(name="osb", bufs=2) as osb,
            tc.tile_pool(name="outsb", bufs=3) as outsb,
            tc.tile_pool(name="small", bufs=4) as small,
            tc.tile_pool(name="dram", bufs=2, space="DRAM") as dramp,
        ):
            # ---- load weights ----
            wk_sb = wpool.tile([128, 6, 128], f32r)   # [ktile-part, ktile, 2x64]
            wv_sb = wpool.tile([128, 6, 128], f32r)
            wq_sb = wpool.tile([128, 6, 192], f32r)
            nc.sync.dma_start(out=wk_sb, in_=wk_l[:, :, :])
            nc.sync.dma_start(out=wv_sb, in_=wv_l[:, :, :])
            nc.sync.dma_start(out=wq_sb, in_=wq_l[:, :, :])
            wo_sb = wpool.tile([64, 3, D], f32r)
            nc.sync.dma_start(out=wo_sb, in_=wo_l[:, :, :])
            ident = wpool.tile([128, 128], f32r)
            nc.sync.dma_start(out=ident, in_=ident_d[:, :])

            # ---- projection phase ----
            KT2 = big.tile([128, N], f32r)       # K^T slot-stacked
            QT01 = big.tile([128, 2048], f32r)
            QT2 = big.tile([64, 2048], f32r)
            V_aug = big.tile([128, NKT, 2, 65], f32r)
            # ones column (softmax denominator accumulator) via host constant
            nc.sync.dma_start(out=V_aug[:, :, :, 64],
                              in_=ones_d[:, :].rearrange("p (a b) -> p a b", a=NKT))
            VT2 = big.tile([128, N], f32r)

            # Projection-phase pools close before the attention pools open:
            # PSUM pools reserve banks statically for their lifetime.
            with (
                tc.tile_pool(name="xchunks", bufs=3) as xchunks,
                tc.tile_pool(name="proj_ps", bufs=2, space="PSUM") as proj_ps,
            ):
                for kc in range(8):
                    xc = xchunks.tile([128, 6, 512], f32r)
                    for kt in range(6):
                        nc.sync.dma_start(
                            out=xc[:, kt, :],
                            in_=xT_r[kt * 128:(kt + 1) * 128, kc * 512:(kc + 1) * 512])
                    ps_k = proj_ps.tile([128, 512], f32, tag="ps_k")
                    ps_v = proj_ps.tile([128, 512], f32, tag="ps_v")
                    ps_q = proj_ps.tile([128, 512], f32, tag="ps_q")
                    for kt in range(6):
                        st, sp = (kt == 0), (kt == 5)
                        nc.tensor.matmul(ps_k, r(wk_sb[:, kt, :]), r(xc[:, kt, :]), start=st, stop=sp)
                        nc.tensor.matmul(ps_v, r(wv_sb[:, kt, :]), r(xc[:, kt, :]), start=st, stop=sp)
                        if kc < 4:
                            nc.tensor.matmul(ps_q, r(wq_sb[:, kt, 0:128]), r(xc[:, kt, :]), start=st, stop=sp)
                        else:
                            nc.tensor.matmul(ps_q[0:64], r(wq_sb[:, kt, 128:192]), r(xc[:, kt, :]), start=st, stop=sp)
                    nc.vector.tensor_copy(KT2[:, kc * 512:(kc + 1) * 512], ps_k)
                    nc.vector.tensor_copy(VT2[:, kc * 512:(kc + 1) * 512], ps_v)
                    if kc < 4:
                        nc.vector.tensor_copy(QT01[:, kc * 512:(kc + 1) * 512], ps_q)
                    else:
                        nc.vector.tensor_copy(QT2[:, (kc - 4) * 512:(kc - 3) * 512], ps_q[0:64])

                # ---- V transpose into natural layout (+ones col stays 1.0) ----
                for kt in range(NKT):
                    ps_t = proj_ps.tile([128, 128], f32r, tag="ps_t")
                    nc.tensor.transpose(ps_t, VT2[:, kt * 128:(kt + 1) * 128], ident)
                    nc.vector.tensor_copy(V_aug[:, kt, 0, 0:64], ps_t[:, 0:64])
                    nc.vector.tensor_copy(V_aug[:, kt, 1, 0:64], ps_t[:, 64:128])

            # ---- attention + out_proj per unit ----
            with (
                tc.tile_pool(name="sc_ps", bufs=2, space="PSUM") as sc_ps,
                tc.tile_pool(name="o_ps", bufs=1, space="PSUM") as o_ps,
                tc.tile_pool(name="op_ps", bufs=1, space="PSUM") as op_ps,
            ):
                ktgs = [(g * KTG, min(KTG, NKT - g * KTG)) for g in range((NKT + KTG - 1) // KTG)]
                O_sbs, recips = [], []
                for j, s in enumerate((0, 1, 0)):
                    QT = QT01[0:64] if j == 0 else (QT01[64:128] if j == 1 else QT2)
                    O_sb = osb.tile([65, 2048], f32r, tag=f"O_sb{min(j, 1)}")
                    for qc in range(NQC):
                        O_ps = o_ps.tile([65, 512], f32, tag="O_ps")
                        first = True
                        for g0, glen in ktgs:
                            sc = sc_ps.tile([128, KTG * 512], f32, tag="sc")
                            for i in range(glen):
                                kt = g0 + i
                                nc.tensor.matmul(
                                    sc[:, i * 512:(i + 1) * 512],
                                    KT2[s * 64:(s + 1) * 64, kt * 128:(kt + 1) * 128],
                                    QT[:, qc * 512:(qc + 1) * 512],
                                    start=True, stop=True)
                            ex = expp.tile([128, KTG * 512], f32r, tag="ex")
                            nc.scalar.activation(
                                ex[:, 0:glen * 512], sc[:, 0:glen * 512],
                                mybir.ActivationFunctionType.Exp, scale=0.125)
                            for i in range(glen):
                                kt = g0 + i
                                nc.tensor.matmul(
                                    O_ps, V_aug[:, kt, s, :], ex[:, i * 512:(i + 1) * 512],
                                    start=first, stop=(kt == NKT - 1))
                                first = False
                        nc.vector.tensor_copy(O_sb[:, qc * 512:(qc + 1) * 512], O_ps)

                    sums_d = dramp.tile([1, 2048], f32, tag="sums_d")
                    nc.sync.dma_start(out=sums_d, in_=O_sb[64:65, :].bitcast(f32))
                    sums_t = small.tile([128, 16], f32, tag=f"sums{min(j, 1)}")
                    nc.sync.dma_start(
                        out=sums_t,
                        in_=sums_d.rearrange("o (t p) -> (o p) t", p=128))
                    recip = small.tile([128, 16], f32, tag=f"recip{min(j, 1)}")
                    nc.vector.reciprocal(recip, sums_t)
                    O_sbs.append(O_sb)
                    recips.append(recip)

                    if j == 0:
                        continue
                    if j == 1:
                        # merged out_proj for U0+U1 (same query rows)
                        pairs = [(O_sbs[0], recips[0], 0), (O_sbs[1], recips[1], 1)]
                        slot = 0
                    else:
                        pairs = [(O_sbs[2], recips[2], 2)]
                        slot = 1
                    for rt in range(16):
                        ob = outsb.tile([128, 768], f32, tag="ob")
                        for pi, (O_u, rc_u, ju) in enumerate(pairs):
                            lhsT = O_u[0:64, rt * 128:(rt + 1) * 128]
                            po1 = op_ps.tile([128, 512], f32, tag="po")
                            nc.tensor.matmul(po1, lhsT, wo_sb[:, ju, 0:512], start=True, stop=True)
                            po2 = op_ps.tile([128, 512], f32, tag="po")
                            nc.tensor.matmul(po2[:, 0:256], lhsT, wo_sb[:, ju, 512:768], start=True, stop=True)
                            if pi == 0:
                                nc.vector.tensor_scalar_mul(ob[:, 0:512], po1, rc_u[:, rt:rt + 1])
                                nc.vector.tensor_scalar_mul(ob[:, 512:768], po2[:, 0:256], rc_u[:, rt:rt + 1])
                            else:
                                tmp = outsb.tile([128, 768], f32, tag="tmp")
                                nc.vector.tensor_scalar_mul(tmp[:, 0:512], po1, rc_u[:, rt:rt + 1])
                                nc.vector.tensor_scalar_mul(tmp[:, 512:768], po2[:, 0:256], rc_u[:, rt:rt + 1])
                                nc.vector.tensor_add(ob, ob, tmp)
                        nc.sync.dma_start(out=out_part[slot, rt * 128:(rt + 1) * 128, :], in_=ob)
    nc.compile()
    return nc


_NC_CACHE = None
_EXEC_CACHE = None


def _install_neff_disk_cache():
    """Persist compiled bass NEFFs across processes (walrus takes minutes)."""
    import hashlib
    import os

    try:
        import libneuronxla
    except ImportError:
        return
    if getattr(libneuronxla, "_bass_neff_disk_cache", False):
        return
    inner = libneuronxla.neuronx_cc
    cachedir = os.path.expanduser("~/.bass_neff_cache")
    os.makedirs(cachedir, exist_ok=True)

    def cached_cc(code, code_format, platform_version, file_prefix):
        if b"bass_exec" not in code:
            return inner(code, code_format, platform_version, file_prefix)
        key = hashlib.sha256(
            repr((code_format, platform_version)).encode() + code).hexdigest()
        path = os.path.join(cachedir, key + ".neff_cc")
        if os.path.exists(path):
            with open(path, "rb") as f:
                return 0, f.read()
        ret = inner(code, code_format, platform_version, file_prefix)
        status, data = ret
        if status == 0:
            tmp = path + ".tmp"
            with open(tmp, "wb") as f:
                f.write(data)
            os.replace(tmp, path)
        return ret

    libneuronxla.neuronx_cc = cached_cc
    libneuronxla._bass_neff_disk_cache = True


def _get_executor():
    """Build (once) a cached sharded jit wrapping the bass NEFF.

    Mirrors concourse.bass2jax.run_bass_via_pjrt but hoists the jitted
    executable into a module-level cache so repeat kernel() calls skip
    retracing/recompiling.
    """
    global _NC_CACHE, _EXEC_CACHE
    if _EXEC_CACHE is not None:
        return _EXEC_CACHE

    import jax
    import concourse.mybir as mybir
    from jax.sharding import Mesh, PartitionSpec
    from jax.experimental.shard_map import shard_map
    from concourse.bass2jax import (
        _bass_exec_p, install_neuronx_cc_hook, partition_id_tensor)

    install_neuronx_cc_hook()
    _install_neff_disk_cache()

    if _NC_CACHE is None:
        _NC_CACHE = _build_bass()
    nc = _NC_CACHE
    partition_name = nc.partition_id_tensor.name if nc.partition_id_tensor else None

    in_names, out_names, out_avals, zero_shapes = [], [], [], []
    for alloc in nc.m.functions[0].allocations:
        if not isinstance(alloc, mybir.MemoryLocationSet):
            continue
        name = alloc.memorylocations[0].name
        if alloc.kind == "ExternalInput":
            if name != partition_name:
                in_names.append(name)
        elif alloc.kind == "ExternalOutput":
            shape = tuple(alloc.tensor_shape)
            dtype = mybir.dt.np(alloc.dtype)
            out_names.append(name)
            out_avals.append(jax.core.ShapedArray(shape, dtype))
            zero_shapes.append((shape, dtype))
    n_params = len(in_names)
    all_names = in_names + out_names
    if partition_name is not None:
        all_names = all_names + [partition_name]

    def _body(*args):
        operands = list(args)
        if partition_name is not None:
            operands.append(partition_id_tensor())
        outs = _bass_exec_p.bind(
            *operands,
            out_avals=tuple(out_avals),
            in_names=tuple(all_names),
            out_names=tuple(out_names),
            lowering_input_output_aliases=(),
            sim_require_finite=True,
            sim_require_nnan=True,
            nc=nc,
        )
        return tuple(outs)

    devices = jax.devices()[:NC]
    mesh = Mesh(np.asarray(devices), ("core",))
    donate = tuple(range(n_params, n_params + len(out_names)))
    sharded = jax.jit(
        shard_map(
            _body, mesh=mesh,
            in_specs=(PartitionSpec("core"),) * (n_params + len(out_names)),
            out_specs=(PartitionSpec("core"),) * len(out_names),
            check_rep=False,
        ),
        donate_argnums=donate, keep_unused=True,
    )

    # Donated output buffers built on-device (no bass_exec -> stock compile
    # path): avoids shipping ~150MB of zeros over the axon tunnel per call.
    import jax.numpy as jnp
    from jax.sharding import NamedSharding

    zero_shardings = tuple(NamedSharding(mesh, PartitionSpec("core"))
                           for _ in zero_shapes)

    @partial(jax.jit, out_shardings=zero_shardings)
    def _make_zeros():
        return tuple(jnp.zeros((NC * s[0], *s[1:]), d) for s, d in zero_shapes)

    _EXEC_CACHE = (sharded, in_names, out_names, out_avals, _make_zeros)
    return _EXEC_CACHE


def kernel(x, w_qkv, w_out, b_out):
    x = np.ascontiguousarray(np.asarray(x, dtype=np.float32))
    w_qkv = np.ascontiguousarray(np.asarray(w_qkv, dtype=np.float32))
    w_out = np.ascontiguousarray(np.asarray(w_out, dtype=np.float32))
    b_out = np.ascontiguousarray(np.asarray(b_out, dtype=np.float32))
    x2 = x[0]

    sharded, in_names, out_names, out_avals, make_zeros = _get_executor()

    in_maps = [_prep_core_inputs(c, x2, w_qkv, w_out) for c in range(NC)]
    concat_in = [
        np.concatenate([in_maps[c][name] for c in range(NC)], axis=0)
        for name in in_names
    ]
    out_arrs = sharded(*concat_in, *make_zeros())

    out = np.zeros((N, D), np.float32)
    parts = np.asarray(out_arrs[out_names.index("out_part")]).reshape(NC, 2, 2048, D)
    for c in range(NC):
        U = _core_units(c)
        out[U[0][1] * 2048:(U[0][1] + 1) * 2048] += parts[c, 0]
        out[U[2][1] * 2048:(U[2][1] + 1) * 2048] += parts[c, 1]
    out += b_out
    return out[None].astype(np.float32)



# revision 4
# speedup vs baseline: 25.8343x; 25.8343x over previous
"""Trainium2 Bass kernel for classical self-attention (B=1, N=4096, D=768, H=12, Hd=64).

Key-sharded flash-style SPMD across 8 NeuronCores, with all data
distribution done ON DEVICE via NeuronLink collectives so only ~18MB
crosses the host<->device tunnel (vs ~220MB for replicated shipping):

  - Core c receives (bf16): x^T columns [512c, 512c+512) (its "local keys"),
    rows [96c, 96c+96) of w_qkv^T and w_out^T, bias, and a 128x128 identity.
  - Device AllGathers w_qkv^T / w_out^T, projects Q/K/V for the local keys,
    AllGathers Q^T so every core has all 4096 queries.
  - Per head: scores^T tiles [128 keys, 512 queries] -> exp (scale=1/8) ->
    PV with a ones-column appended to V so the softmax denominator
    accumulates for free in row 64 of the O^T PSUM tile.
  - O^T tiles are PE-transposed into a q-major partial-numerator DRAM
    tensor [8, 512, 784] f32 (cols 768:780 hold the 12 per-head denominators)
    and ReduceScattered: core c ends up with the fully-summed numerator for
    queries [512c, 512c+512).
  - Normalize per (query, head), PE-transpose, out_proj with the bias folded
    in as a ones-row matmul, emit the final [512, 768] bf16 slice.

Host does only casts/reshapes; outputs concatenate directly to [4096, 768].
"""
import numpy as np
from functools import partial

H, Hd, N, D = 12, 64, 4096, 768
NC = 8
KL = N // NC          # 512 local keys per core
QL = N // NC          # 512 output query rows per core
NP = D + 16           # packed numerator width (768 num + 12 den + 4 pad)


def _build_bass():
    import concourse.mybir as mybir
    import concourse.tile as tile
    from concourse import bacc

    f32 = mybir.dt.float32
    f32r = mybir.dt.float32r
    bf16 = mybir.dt.bfloat16
    Exp = mybir.ActivationFunctionType.Exp
    nc = bacc.Bacc(None, target_bir_lowering=False, num_devices=NC)
    RG = [list(range(NC))]

    xTc = nc.dram_tensor("xTc", [D, KL], bf16, kind="ExternalInput")
    wqkvTc = nc.dram_tensor("wqkvTc", [D // NC, 3 * D], bf16, kind="ExternalInput")
    woTc = nc.dram_tensor("woTc", [D // NC, D], bf16, kind="ExternalInput")
    biasc = nc.dram_tensor("biasc", [1, D], bf16, kind="ExternalInput")
    identc = nc.dram_tensor("identc", [128, 128], bf16, kind="ExternalInput")
    outc = nc.dram_tensor("outc", [QL, D], bf16, kind="ExternalOutput")

    wq_st = nc.dram_tensor("wq_st", [D // NC, 3 * D], bf16, kind="Internal")
    wo_st = nc.dram_tensor("wo_st", [D // NC, D], bf16, kind="Internal")
    wqkvT_g = nc.dram_tensor("wqkvT_g", [D, 3 * D], bf16, kind="Internal",
                             addr_space="Shared")
    woT_g = nc.dram_tensor("woT_g", [D, D], bf16, kind="Internal",
                           addr_space="Shared")
    q_st = nc.dram_tensor("q_st", [6, 128, KL], bf16, kind="Internal")
    qT_g = nc.dram_tensor("qT_g", [NC, 6, 128, KL], bf16, kind="Internal",
                          addr_space="Shared")
    num_p = nc.dram_tensor("num_p", [NC, QL, NP], f32, kind="Internal")
    num_rs = nc.dram_tensor("num_rs", [QL, NP], f32, kind="Internal")

    with tile.TileContext(nc) as tc:
        with (
            tc.tile_pool(name="wpool", bufs=1) as wpool,
            tc.tile_pool(name="big", bufs=1) as big,
            tc.tile_pool(name="stage", bufs=2) as stage,
        ):
            x_sb = big.tile([128, 6, KL], bf16)
            for t in range(6):
                nc.sync.dma_start(out=x_sb[:, t, :], in_=xTc[t * 128:(t + 1) * 128, :])
            ident = wpool.tile([128, 128], bf16)
            nc.sync.dma_start(out=ident, in_=identc[:, :])
            ident_f = wpool.tile([128, 128], f32r)
            nc.vector.tensor_copy(ident_f, ident)
            bias_sb = wpool.tile([1, D], bf16)
            nc.sync.dma_start(out=bias_sb, in_=biasc[:, :])
            ones_row = wpool.tile([1, 128], bf16)
            nc.vector.memset(ones_row, 1.0)

            # stage weights through Internal DRAM, AllGather over NeuronLink
            wst_sb = stage.tile([D // NC, 3 * D], bf16, tag="wst")
            nc.sync.dma_start(out=wst_sb, in_=wqkvTc[:, :])
            nc.sync.dma_start(out=wq_st[:, :], in_=wst_sb)
            nc.gpsimd.collective_compute(
                "AllGather", mybir.AluOpType.bypass, replica_groups=RG,
                ins=[wq_st[:, :]], outs=[wqkvT_g[:, :]])
            wost_sb = stage.tile([D // NC, D], bf16, tag="wost")
            nc.sync.dma_start(out=wost_sb, in_=woTc[:, :])
            nc.sync.dma_start(out=wo_st[:, :], in_=wost_sb)
            nc.gpsimd.collective_compute(
                "AllGather", mybir.AluOpType.bypass, replica_groups=RG,
                ins=[wo_st[:, :]], outs=[woT_g[:, :]])

            wqkv_sb = wpool.tile([128, 6, 3 * D], bf16)
            for t in range(6):
                nc.sync.dma_start(out=wqkv_sb[:, t, :],
                                  in_=wqkvT_g[t * 128:(t + 1) * 128, :])
            wo_sb = wpool.tile([128, 6, D], bf16)
            for t in range(6):
                nc.sync.dma_start(out=wo_sb[:, t, :],
                                  in_=woT_g[t * 128:(t + 1) * 128, :])

            kT_sb = big.tile([128, 6, KL], bf16)
            vT_sb = big.tile([128, 6, KL], bf16)
            V_aug = big.tile([128, 4, H, Hd + 1], bf16)
            nc.vector.memset(V_aug[:, :, :, Hd], 1.0)

            # ---- QKV projection for local keys (contraction over d) ----
            with (
                tc.tile_pool(name="qtmp", bufs=3) as qtmp,
                tc.tile_pool(name="proj_ps", bufs=3, space="PSUM") as proj_ps,
            ):
                for jb in range(18):
                    ps = proj_ps.tile([128, KL], f32, tag="ps")
                    for t in range(6):
                        nc.tensor.matmul(ps, wqkv_sb[:, t, jb * 128:(jb + 1) * 128],
                                         x_sb[:, t, :], start=(t == 0), stop=(t == 5))
                    if jb < 6:
                        q_sb = qtmp.tile([128, KL], bf16, tag="q")
                        nc.vector.tensor_copy(q_sb, ps)
                        nc.sync.dma_start(out=q_st[jb, :, :], in_=q_sb)
                    elif jb < 12:
                        nc.vector.tensor_copy(kT_sb[:, jb - 6, :], ps)
                    else:
                        nc.vector.tensor_copy(vT_sb[:, jb - 12, :], ps)
                nc.gpsimd.collective_compute(
                    "AllGather", mybir.AluOpType.bypass, replica_groups=RG,
                    ins=[q_st[:, :, :]], outs=[qT_g[:, :, :, :]])
                # V^T -> natural key-major layout (+ones column stays 1.0)
                for h in range(H):
                    po = (h % 2) * 64
                    for kt in range(4):
                        pt = proj_ps.tile([128, Hd], bf16, tag="pt")
                        nc.tensor.transpose(
                            pt, vT_sb[po:po + 64, h // 2, kt * 128:(kt + 1) * 128],
                            ident[po:po + 64, po:po + 64])
                        nc.vector.tensor_copy(V_aug[:, kt, h, 0:Hd], pt)

            # ---- attention: all queries x local keys, per head ----
            with (
                tc.tile_pool(name="qbp", bufs=2) as qbp,
                tc.tile_pool(name="expp", bufs=3) as expp,
                tc.tile_pool(name="osbp", bufs=2) as osbp,
                tc.tile_pool(name="numpool", bufs=2) as numpool,
                tc.tile_pool(name="sc_ps", bufs=2, space="PSUM") as sc_ps,
                tc.tile_pool(name="o_ps", bufs=2, space="PSUM") as o_ps,
                tc.tile_pool(name="tp_ps", bufs=2, space="PSUM") as tp_ps,
            ):
                for b in range(NC):
                    qb_sb = qbp.tile([128, 6, KL], bf16, tag="qb")
                    for t in range(6):
                        nc.sync.dma_start(out=qb_sb[:, t, :], in_=qT_g[b, t, :, :])
                    num_sb = numpool.tile([128, 4, NP], f32, tag="num")
                    for h in range(H):
                        po = (h % 2) * 64
                        o_psum = o_ps.tile([Hd + 1, KL], f32, tag="o")
                        for g in range(2):
                            sc = sc_ps.tile([128, 2, KL], f32, tag="sc")
                            for i in range(2):
                                kt = g * 2 + i
                                nc.tensor.matmul(
                                    sc[:, i, :],
                                    kT_sb[po:po + 64, h // 2, kt * 128:(kt + 1) * 128],
                                    qb_sb[po:po + 64, h // 2, :],
                                    start=True, stop=True)
                            ex = expp.tile([128, 2, KL], bf16, tag="ex")
                            nc.scalar.activation(ex[:, :, :], sc[:, :, :], Exp,
                                                 scale=0.125)
                            for i in range(2):
                                kt = g * 2 + i
                                nc.tensor.matmul(o_psum, V_aug[:, kt, h, :],
                                                 ex[:, i, :],
                                                 start=(kt == 0), stop=(kt == 3))
                        o_sb = osbp.tile([Hd + 1, KL], f32r, tag="ot")
                        nc.vector.tensor_copy(o_sb, o_psum)
                        for qs in range(4):
                            pt = tp_ps.tile([128, Hd + 2], f32r, tag="pt2")
                            nc.tensor.transpose(
                                pt, o_sb[:, qs * 128:(qs + 1) * 128],
                                ident_f[0:Hd + 1, 0:Hd + 2])
                            nc.vector.tensor_copy(
                                num_sb[:, qs, h * 64:(h + 1) * 64], pt[:, 0:Hd])
                            nc.vector.tensor_copy(
                                num_sb[:, qs, D + h:D + h + 1], pt[:, Hd:Hd + 1])
                    nc.sync.dma_start(
                        out=num_p[b, :, :].rearrange("(qs p) i -> p qs i", p=128),
                        in_=num_sb)
                nc.gpsimd.collective_compute(
                    "ReduceScatter", mybir.AluOpType.add, replica_groups=RG,
                    ins=[num_p[:, :, :]], outs=[num_rs[:, :]])

            # ---- finalize: normalize + out_proj (+bias) for own q-slice ----
            with (
                tc.tile_pool(name="fin", bufs=1) as fin,
                tc.tile_pool(name="outsb", bufs=2) as outsb,
                tc.tile_pool(name="fps", bufs=2, space="PSUM") as fps,
            ):
                nfin = fin.tile([128, 4, NP], f32)
                nc.sync.dma_start(
                    out=nfin, in_=num_rs[:, :].rearrange("(qs p) i -> p qs i", p=128))
                rec = fin.tile([128, 4, H], f32)
                nc.vector.reciprocal(rec, nfin[:, :, D:D + H])
                nn_sb = fin.tile([128, 4, D], bf16)
                for qs in range(4):
                    for h in range(H):
                        nc.vector.tensor_scalar_mul(
                            nn_sb[:, qs, h * 64:(h + 1) * 64],
                            nfin[:, qs, h * 64:(h + 1) * 64],
                            rec[:, qs, h:h + 1])
                nT_sb = fin.tile([128, 6, 4, 128], bf16)
                for qs in range(4):
                    for ic in range(6):
                        pt2 = fps.tile([128, 128], bf16, tag="pt3")
                        nc.tensor.transpose(
                            pt2, nn_sb[:, qs, ic * 128:(ic + 1) * 128], ident)
                        nc.vector.tensor_copy(nT_sb[:, ic, qs, :], pt2)
                for qs in range(4):
                    po1 = fps.tile([128, 512], f32, tag="po1")
                    po2 = fps.tile([128, 256], f32, tag="po2")
                    for ic in range(6):
                        nc.tensor.matmul(po1, nT_sb[:, ic, qs, :],
                                         wo_sb[:, ic, 0:512],
                                         start=(ic == 0), stop=False)
                        nc.tensor.matmul(po2, nT_sb[:, ic, qs, :],
                                         wo_sb[:, ic, 512:768],
                                         start=(ic == 0), stop=False)
                    nc.tensor.matmul(po1, ones_row, bias_sb[0:1, 0:512],
                                     start=False, stop=True)
                    nc.tensor.matmul(po2, ones_row, bias_sb[0:1, 512:768],
                                     start=False, stop=True)
                    ob = outsb.tile([128, D], bf16, tag="ob")
                    nc.vector.tensor_copy(ob[:, 0:512], po1)
                    nc.vector.tensor_copy(ob[:, 512:768], po2)
                    nc.sync.dma_start(out=outc[qs * 128:(qs + 1) * 128, :], in_=ob)
    nc.compile()
    return nc


_NC_CACHE = None
_EXEC_CACHE = None


def _install_neff_disk_cache():
    """Persist compiled bass NEFFs across processes (walrus takes minutes)."""
    import hashlib
    import os

    try:
        import libneuronxla
    except ImportError:
        return
    if getattr(libneuronxla, "_bass_neff_disk_cache", False):
        return
    inner = libneuronxla.neuronx_cc
    cachedir = os.path.expanduser("~/.bass_neff_cache")
    os.makedirs(cachedir, exist_ok=True)

    def cached_cc(code, code_format, platform_version, file_prefix):
        if b"bass_exec" not in code:
            return inner(code, code_format, platform_version, file_prefix)
        key = hashlib.sha256(
            repr((code_format, platform_version)).encode() + code).hexdigest()
        path = os.path.join(cachedir, key + ".neff_cc")
        if os.path.exists(path):
            with open(path, "rb") as f:
                return 0, f.read()
        ret = inner(code, code_format, platform_version, file_prefix)
        status, data = ret
        if status == 0:
            tmp = path + ".tmp"
            with open(tmp, "wb") as f:
                f.write(data)
            os.replace(tmp, path)
        return ret

    libneuronxla.neuronx_cc = cached_cc
    libneuronxla._bass_neff_disk_cache = True


def _get_executor():
    """Build (once) a cached sharded jit wrapping the bass NEFF."""
    global _NC_CACHE, _EXEC_CACHE
    if _EXEC_CACHE is not None:
        return _EXEC_CACHE

    import jax
    import concourse.mybir as mybir
    from jax.sharding import Mesh, PartitionSpec
    from jax.experimental.shard_map import shard_map
    from concourse.bass2jax import (
        _bass_exec_p, install_neuronx_cc_hook, partition_id_tensor)

    install_neuronx_cc_hook()
    _install_neff_disk_cache()

    if _NC_CACHE is None:
        _NC_CACHE = _build_bass()
    nc = _NC_CACHE
    partition_name = nc.partition_id_tensor.name if nc.partition_id_tensor else None

    in_names, out_names, out_avals, zero_shapes = [], [], [], []
    for alloc in nc.m.functions[0].allocations:
        if not isinstance(alloc, mybir.MemoryLocationSet):
            continue
        name = alloc.memorylocations[0].name
        if alloc.kind == "ExternalInput":
            if name != partition_name:
                in_names.append(name)
        elif alloc.kind == "ExternalOutput":
            shape = tuple(alloc.tensor_shape)
            dtype = mybir.dt.np(alloc.dtype)
            out_names.append(name)
            out_avals.append(jax.core.ShapedArray(shape, dtype))
            zero_shapes.append((shape, dtype))
    n_params = len(in_names)
    all_names = in_names + out_names
    if partition_name is not None:
        all_names = all_names + [partition_name]

    def _body(*args):
        operands = list(args)
        if partition_name is not None:
            operands.append(partition_id_tensor())
        outs = _bass_exec_p.bind(
            *operands,
            out_avals=tuple(out_avals),
            in_names=tuple(all_names),
            out_names=tuple(out_names),
            lowering_input_output_aliases=(),
            sim_require_finite=True,
            sim_require_nnan=True,
            nc=nc,
        )
        return tuple(outs)

    devices = jax.devices()[:NC]
    mesh = Mesh(np.asarray(devices), ("core",))
    donate = tuple(range(n_params, n_params + len(out_names)))
    sharded = jax.jit(
        shard_map(
            _body, mesh=mesh,
            in_specs=(PartitionSpec("core"),) * (n_params + len(out_names)),
            out_specs=(PartitionSpec("core"),) * len(out_names),
            check_rep=False,
        ),
        donate_argnums=donate, keep_unused=True,
    )

    # Donated output buffers built on-device: nothing shipped over the tunnel.
    import jax.numpy as jnp
    from jax.sharding import NamedSharding

    zero_shardings = tuple(NamedSharding(mesh, PartitionSpec("core"))
                           for _ in zero_shapes)

    @partial(jax.jit, out_shardings=zero_shardings)
    def _make_zeros():
        return tuple(jnp.zeros((NC * s[0], *s[1:]), d) for s, d in zero_shapes)

    _EXEC_CACHE = (sharded, in_names, out_names, out_avals, _make_zeros)
    return _EXEC_CACHE


_IDENT_STACK = None


def kernel(x, w_qkv, w_out, b_out):
    global _IDENT_STACK
    import ml_dtypes
    bf = ml_dtypes.bfloat16

    sharded, in_names, out_names, out_avals, make_zeros = _get_executor()

    x2 = np.asarray(x, dtype=np.float32).reshape(N, D)
    # per-core x^T column blocks, stacked: [8*768, 512]
    x_stack = x2.reshape(NC, KL, D).transpose(0, 2, 1).astype(bf).reshape(NC * D, KL)
    wqkvT = np.asarray(w_qkv, np.float32).T.astype(bf)       # [768, 2304]
    woT = np.asarray(w_out, np.float32).T.astype(bf)         # [768, 768]
    bias = np.broadcast_to(np.asarray(b_out, np.float32).astype(bf), (NC, D))
    if _IDENT_STACK is None:
        _IDENT_STACK = np.tile(np.eye(128, dtype=bf), (NC, 1))

    in_map = {"xTc": x_stack, "wqkvTc": wqkvT, "woTc": woT,
              "biasc": bias, "identc": _IDENT_STACK}
    out_arrs = sharded(*[in_map[n] for n in in_names], *make_zeros())
    out = np.asarray(out_arrs[out_names.index("outc")])      # [4096, 768] bf16
    return out.astype(np.float32)[None]


# revision 10
# speedup vs baseline: 36.6085x; 1.4170x over previous
"""Trainium2 Bass kernel for classical self-attention (B=1, N=4096, D=768, H=12, Hd=64).

Key-sharded flash-style SPMD across 8 NeuronCores, with all data
distribution done ON DEVICE via NeuronLink collectives so only ~18MB
crosses the host<->device tunnel (vs ~220MB for replicated shipping):

  - Core c receives (bf16): x^T columns [512c, 512c+512) (its "local keys"),
    rows [96c, 96c+96) of w_qkv^T and w_out^T, bias, and a 128x128 identity.
  - Device AllGathers w_qkv^T / w_out^T, projects Q/K/V for the local keys,
    AllGathers Q^T so every core has all 4096 queries.
  - Per head: scores^T tiles [128 keys, 512 queries] -> exp (scale=1/8) ->
    PV with a ones-column appended to V so the softmax denominator
    accumulates for free in row 64 of the O^T PSUM tile.
  - O^T tiles are PE-transposed into a q-major partial-numerator DRAM
    tensor [8, 512, 784] f32 (cols 768:780 hold the 12 per-head denominators)
    and ReduceScattered: core c ends up with the fully-summed numerator for
    queries [512c, 512c+512).
  - Normalize per (query, head), PE-transpose, out_proj with the bias folded
    in as a ones-row matmul, emit the final [512, 768] bf16 slice.

Host does only casts/reshapes; outputs concatenate directly to [4096, 768].
"""
import numpy as np
from functools import partial

H, Hd, N, D = 12, 64, 4096, 768
NC = 8
KL = N // NC          # 512 local keys per core
QL = N // NC          # 512 output query rows per core
NP = D + 16           # packed numerator width (768 num + 12 den + 4 pad)


def _build_bass():
    import concourse.mybir as mybir
    import concourse.tile as tile
    from concourse import bacc

    f32 = mybir.dt.float32
    f32r = mybir.dt.float32r
    bf16 = mybir.dt.bfloat16
    Exp = mybir.ActivationFunctionType.Exp
    nc = bacc.Bacc(None, target_bir_lowering=False, num_devices=NC)
    RG = [list(range(NC))]

    xTc = nc.dram_tensor("xTc", [D, KL], bf16, kind="ExternalInput")
    wqkvTc = nc.dram_tensor("wqkvTc", [D // NC, 3 * D], bf16, kind="ExternalInput")
    woTc = nc.dram_tensor("woTc", [D // NC, D], bf16, kind="ExternalInput")
    biasc = nc.dram_tensor("biasc", [1, D], bf16, kind="ExternalInput")
    identc = nc.dram_tensor("identc", [128, 128], bf16, kind="ExternalInput")
    outc = nc.dram_tensor("outc", [QL, D], bf16, kind="ExternalOutput")

    wq_st = nc.dram_tensor("wq_st", [D // NC, 3 * D], bf16, kind="Internal")
    wo_st = nc.dram_tensor("wo_st", [D // NC, D], bf16, kind="Internal")
    wqkvT_g = nc.dram_tensor("wqkvT_g", [D, 3 * D], bf16, kind="Internal",
                             addr_space="Shared")
    woT_g = nc.dram_tensor("woT_g", [D, D], bf16, kind="Internal",
                           addr_space="Shared")
    q_st = nc.dram_tensor("q_st", [6, 128, KL], bf16, kind="Internal")
    qT_g = nc.dram_tensor("qT_g", [NC, 6, 128, KL], bf16, kind="Internal",
                          addr_space="Shared")
    num_p = nc.dram_tensor("num_p", [NC, QL, NP], f32, kind="Internal")
    num_rs = nc.dram_tensor("num_rs", [QL, NP], f32, kind="Internal")

    with tile.TileContext(nc) as tc:
        with (
            tc.tile_pool(name="wpool", bufs=1) as wpool,
            tc.tile_pool(name="big", bufs=1) as big,
            tc.tile_pool(name="stage", bufs=2) as stage,
        ):
            x_sb = big.tile([128, 6, KL], bf16)
            for t in range(6):
                nc.sync.dma_start(out=x_sb[:, t, :], in_=xTc[t * 128:(t + 1) * 128, :])
            ident = wpool.tile([128, 128], bf16)
            nc.sync.dma_start(out=ident, in_=identc[:, :])
            ident_f = wpool.tile([128, 128], f32r)
            nc.vector.tensor_copy(ident_f, ident)
            bias_sb = wpool.tile([1, D], bf16)
            nc.sync.dma_start(out=bias_sb, in_=biasc[:, :])
            ones_row = wpool.tile([1, 128], bf16)
            nc.vector.memset(ones_row, 1.0)

            # stage weights through Internal DRAM, AllGather over NeuronLink
            wst_sb = stage.tile([D // NC, 3 * D], bf16, tag="wst")
            nc.sync.dma_start(out=wst_sb, in_=wqkvTc[:, :])
            nc.sync.dma_start(out=wq_st[:, :], in_=wst_sb)
            nc.gpsimd.collective_compute(
                "AllGather", mybir.AluOpType.bypass, replica_groups=RG,
                ins=[wq_st[:, :]], outs=[wqkvT_g[:, :]])
            wost_sb = stage.tile([D // NC, D], bf16, tag="wost")
            nc.sync.dma_start(out=wost_sb, in_=woTc[:, :])
            nc.sync.dma_start(out=wo_st[:, :], in_=wost_sb)
            nc.gpsimd.collective_compute(
                "AllGather", mybir.AluOpType.bypass, replica_groups=RG,
                ins=[wo_st[:, :]], outs=[woT_g[:, :]])

            wqkv_sb = wpool.tile([128, 6, 3 * D], bf16)
            for t in range(6):
                nc.sync.dma_start(out=wqkv_sb[:, t, :],
                                  in_=wqkvT_g[t * 128:(t + 1) * 128, :])
            wo_sb = wpool.tile([128, 6, D], bf16)
            for t in range(6):
                nc.sync.dma_start(out=wo_sb[:, t, :],
                                  in_=woT_g[t * 128:(t + 1) * 128, :])

            kT_sb = big.tile([128, 6, KL], bf16)
            vT_sb = big.tile([128, 6, KL], bf16)
            V_aug = big.tile([128, 4, H, Hd + 1], bf16)
            nc.vector.memset(V_aug[:, :, :, Hd], 1.0)

            # ---- QKV projection for local keys (contraction over d) ----
            with (
                tc.tile_pool(name="qtmp", bufs=3) as qtmp,
                tc.tile_pool(name="proj_ps", bufs=3, space="PSUM") as proj_ps,
            ):
                for jb in range(18):
                    ps = proj_ps.tile([128, KL], f32, tag="ps")
                    for t in range(6):
                        nc.tensor.matmul(ps, wqkv_sb[:, t, jb * 128:(jb + 1) * 128],
                                         x_sb[:, t, :], start=(t == 0), stop=(t == 5))
                    if jb < 6:
                        q_sb = qtmp.tile([128, KL], bf16, tag="q")
                        nc.vector.tensor_copy(q_sb, ps)
                        nc.sync.dma_start(out=q_st[jb, :, :], in_=q_sb)
                    elif jb < 12:
                        nc.vector.tensor_copy(kT_sb[:, jb - 6, :], ps)
                    else:
                        nc.vector.tensor_copy(vT_sb[:, jb - 12, :], ps)
                nc.gpsimd.collective_compute(
                    "AllGather", mybir.AluOpType.bypass, replica_groups=RG,
                    ins=[q_st[:, :, :]], outs=[qT_g[:, :, :, :]])
                # V^T -> natural key-major layout (+ones column stays 1.0)
                for h in range(H):
                    po = (h % 2) * 64
                    for kt in range(4):
                        pt = proj_ps.tile([128, Hd], bf16, tag="pt")
                        nc.tensor.transpose(
                            pt, vT_sb[po:po + 64, h // 2, kt * 128:(kt + 1) * 128],
                            ident[po:po + 64, po:po + 64])
                        nc.vector.tensor_copy(V_aug[:, kt, h, 0:Hd], pt)

            # ---- attention: all queries x local keys, per head ----
            with (
                tc.tile_pool(name="qbp", bufs=2) as qbp,
                tc.tile_pool(name="expp", bufs=3) as expp,
                tc.tile_pool(name="osbp", bufs=2) as osbp,
                tc.tile_pool(name="numpool", bufs=2) as numpool,
                tc.tile_pool(name="sc_ps", bufs=2, space="PSUM") as sc_ps,
                tc.tile_pool(name="o_ps", bufs=2, space="PSUM") as o_ps,
                tc.tile_pool(name="tp_ps", bufs=2, space="PSUM") as tp_ps,
            ):
                for b in range(NC):
                    qb_sb = qbp.tile([128, 6, KL], bf16, tag="qb")
                    for t in range(6):
                        nc.sync.dma_start(out=qb_sb[:, t, :], in_=qT_g[b, t, :, :])
                    num_sb = numpool.tile([128, 4, NP], f32, tag="num")
                    for h in range(H):
                        po = (h % 2) * 64
                        o_psum = o_ps.tile([Hd + 1, KL], f32, tag="o")
                        for g in range(2):
                            sc = sc_ps.tile([128, 2, KL], f32, tag="sc")
                            for i in range(2):
                                kt = g * 2 + i
                                nc.tensor.matmul(
                                    sc[:, i, :],
                                    kT_sb[po:po + 64, h // 2, kt * 128:(kt + 1) * 128],
                                    qb_sb[po:po + 64, h // 2, :],
                                    start=True, stop=True)
                            ex = expp.tile([128, 2, KL], bf16, tag="ex")
                            nc.scalar.activation(ex[:, :, :], sc[:, :, :], Exp,
                                                 scale=0.125)
                            for i in range(2):
                                kt = g * 2 + i
                                nc.tensor.matmul(o_psum, V_aug[:, kt, h, :],
                                                 ex[:, i, :],
                                                 start=(kt == 0), stop=(kt == 3))
                        o_sb = osbp.tile([Hd + 1, KL], f32r, tag="ot")
                        nc.vector.tensor_copy(o_sb, o_psum)
                        for qs in range(4):
                            pt = tp_ps.tile([128, Hd + 2], f32r, tag="pt2")
                            nc.tensor.transpose(
                                pt, o_sb[:, qs * 128:(qs + 1) * 128],
                                ident_f[0:Hd + 1, 0:Hd + 2])
                            nc.vector.tensor_copy(
                                num_sb[:, qs, h * 64:(h + 1) * 64], pt[:, 0:Hd])
                            nc.vector.tensor_copy(
                                num_sb[:, qs, D + h:D + h + 1], pt[:, Hd:Hd + 1])
                    nc.sync.dma_start(
                        out=num_p[b, :, :].rearrange("(qs p) i -> p qs i", p=128),
                        in_=num_sb)
                nc.gpsimd.collective_compute(
                    "ReduceScatter", mybir.AluOpType.add, replica_groups=RG,
                    ins=[num_p[:, :, :]], outs=[num_rs[:, :]])

            # ---- finalize: normalize + out_proj (+bias) for own q-slice ----
            with (
                tc.tile_pool(name="fin", bufs=1) as fin,
                tc.tile_pool(name="outsb", bufs=2) as outsb,
                tc.tile_pool(name="fps", bufs=2, space="PSUM") as fps,
            ):
                nfin = fin.tile([128, 4, NP], f32)
                nc.sync.dma_start(
                    out=nfin, in_=num_rs[:, :].rearrange("(qs p) i -> p qs i", p=128))
                rec = fin.tile([128, 4, H], f32)
                nc.vector.reciprocal(rec, nfin[:, :, D:D + H])
                nn_sb = fin.tile([128, 4, D], bf16)
                for qs in range(4):
                    for h in range(H):
                        nc.vector.tensor_scalar_mul(
                            nn_sb[:, qs, h * 64:(h + 1) * 64],
                            nfin[:, qs, h * 64:(h + 1) * 64],
                            rec[:, qs, h:h + 1])
                nT_sb = fin.tile([128, 6, 4, 128], bf16)
                for qs in range(4):
                    for ic in range(6):
                        pt2 = fps.tile([128, 128], bf16, tag="pt3")
                        nc.tensor.transpose(
                            pt2, nn_sb[:, qs, ic * 128:(ic + 1) * 128], ident)
                        nc.vector.tensor_copy(nT_sb[:, ic, qs, :], pt2)
                for qs in range(4):
                    po1 = fps.tile([128, 512], f32, tag="po1")
                    po2 = fps.tile([128, 256], f32, tag="po2")
                    for ic in range(6):
                        nc.tensor.matmul(po1, nT_sb[:, ic, qs, :],
                                         wo_sb[:, ic, 0:512],
                                         start=(ic == 0), stop=False)
                        nc.tensor.matmul(po2, nT_sb[:, ic, qs, :],
                                         wo_sb[:, ic, 512:768],
                                         start=(ic == 0), stop=False)
                    nc.tensor.matmul(po1, ones_row, bias_sb[0:1, 0:512],
                                     start=False, stop=True)
                    nc.tensor.matmul(po2, ones_row, bias_sb[0:1, 512:768],
                                     start=False, stop=True)
                    ob = outsb.tile([128, D], bf16, tag="ob")
                    nc.vector.tensor_copy(ob[:, 0:512], po1)
                    nc.vector.tensor_copy(ob[:, 512:768], po2)
                    nc.sync.dma_start(out=outc[qs * 128:(qs + 1) * 128, :], in_=ob)
    nc.compile()
    return nc


_NC_CACHE = None
_EXEC_CACHE = None


def _install_neff_disk_cache():
    """Persist compiled bass NEFFs across processes (walrus takes minutes)."""
    import hashlib
    import os

    try:
        import libneuronxla
    except ImportError:
        return
    if getattr(libneuronxla, "_bass_neff_disk_cache", False):
        return
    inner = libneuronxla.neuronx_cc
    cachedir = os.path.expanduser("~/.bass_neff_cache")
    os.makedirs(cachedir, exist_ok=True)

    def cached_cc(code, code_format, platform_version, file_prefix):
        if b"bass_exec" not in code:
            return inner(code, code_format, platform_version, file_prefix)
        key = hashlib.sha256(
            repr((code_format, platform_version)).encode() + code).hexdigest()
        path = os.path.join(cachedir, key + ".neff_cc")
        if os.path.exists(path):
            with open(path, "rb") as f:
                return 0, f.read()
        ret = inner(code, code_format, platform_version, file_prefix)
        status, data = ret
        if status == 0:
            tmp = path + ".tmp"
            with open(tmp, "wb") as f:
                f.write(data)
            os.replace(tmp, path)
        return ret

    libneuronxla.neuronx_cc = cached_cc
    libneuronxla._bass_neff_disk_cache = True


def _get_executor():
    """Build (once) a cached sharded jit wrapping the bass NEFF."""
    global _NC_CACHE, _EXEC_CACHE
    if _EXEC_CACHE is not None:
        return _EXEC_CACHE

    import jax
    import concourse.mybir as mybir
    from jax.sharding import Mesh, PartitionSpec
    from jax.experimental.shard_map import shard_map
    from concourse.bass2jax import (
        _bass_exec_p, install_neuronx_cc_hook, partition_id_tensor)

    install_neuronx_cc_hook()
    _install_neff_disk_cache()

    if _NC_CACHE is None:
        _NC_CACHE = _build_bass()
    nc = _NC_CACHE
    partition_name = nc.partition_id_tensor.name if nc.partition_id_tensor else None

    in_names, out_names, out_avals, zero_shapes = [], [], [], []
    for alloc in nc.m.functions[0].allocations:
        if not isinstance(alloc, mybir.MemoryLocationSet):
            continue
        name = alloc.memorylocations[0].name
        if alloc.kind == "ExternalInput":
            if name != partition_name:
                in_names.append(name)
        elif alloc.kind == "ExternalOutput":
            shape = tuple(alloc.tensor_shape)
            dtype = mybir.dt.np(alloc.dtype)
            out_names.append(name)
            out_avals.append(jax.core.ShapedArray(shape, dtype))
            zero_shapes.append((shape, dtype))
    n_params = len(in_names)
    all_names = in_names + out_names
    if partition_name is not None:
        all_names = all_names + [partition_name]

    import jax.numpy as jnp
    from jax.sharding import NamedSharding

    def _body(*args):
        operands = list(args)
        if partition_name is not None:
            operands.append(partition_id_tensor())
        outs = _bass_exec_p.bind(
            *operands,
            out_avals=tuple(out_avals),
            in_names=tuple(all_names),
            out_names=tuple(out_names),
            lowering_input_output_aliases=(),
            sim_require_finite=True,
            sim_require_nnan=True,
            nc=nc,
        )
        return tuple(outs)

    devices = jax.devices()[:NC]
    mesh = Mesh(np.asarray(devices), ("core",))
    donate = tuple(range(n_params, n_params + len(out_names)))
    sharded = jax.jit(
        shard_map(
            _body, mesh=mesh,
            in_specs=(PartitionSpec("core"),) * (n_params + len(out_names)),
            out_specs=(PartitionSpec("core"),) * len(out_names),
            check_rep=False,
        ),
        donate_argnums=donate, keep_unused=True,
    )

    in_sharding = NamedSharding(mesh, PartitionSpec("core"))
    zero_shardings = tuple(in_sharding for _ in zero_shapes)

    @partial(jax.jit, out_shardings=zero_shardings)
    def _make_zeros():
        return tuple(jnp.zeros((NC * s[0], *s[1:]), d) for s, d in zero_shapes)

    _EXEC_CACHE = (sharded, in_names, out_names, out_avals, in_sharding,
                   _make_zeros)
    return _EXEC_CACHE


# Weights are shipped to the device once and reused while the kernel()
# weight arguments stay bit-identical (memcmp ~5ms vs re-shipping ~5MB
# over the ~35MB/s tunnel). Activations (x) are shipped every call.
_WCACHE = {}
# The bass program fully overwrites its output tensors, so the donated
# output buffers' content is irrelevant: steady-state calls donate the
# PREVIOUS call's (already host-copied) output buffers instead of paying
# an extra make_zeros dispatch round trip.
_DONATE_NEXT = None


def kernel(x, w_qkv, w_out, b_out):
    global _DONATE_NEXT
    import ml_dtypes
    bf = ml_dtypes.bfloat16

    sharded, in_names, out_names, out_avals, in_sharding, make_zeros = \
        _get_executor()

    x2 = np.asarray(x, dtype=np.float32).reshape(N, D)
    # per-core x^T column blocks, stacked: [8*768, 512]
    x_stack = x2.reshape(NC, KL, D).transpose(0, 2, 1).astype(bf).reshape(NC * D, KL)

    w_qkv = np.asarray(w_qkv, np.float32)
    w_out = np.asarray(w_out, np.float32)
    b_out = np.asarray(b_out, np.float32)
    ent = _WCACHE.get("w")
    if ent is not None and all(
            k is c or (k.shape == c.shape and np.array_equal(k, c))
            for k, c in zip((w_qkv, w_out, b_out), ent[0])):
        wqkvT_d, woT_d, bias_d, ident_d = ent[1]
    else:
        import jax
        wqkvT = w_qkv.T.astype(bf)                            # [768, 2304]
        woT = w_out.T.astype(bf)                              # [768, 768]
        bias = np.broadcast_to(b_out.astype(bf), (NC, D))
        ident = np.tile(np.eye(128, dtype=bf), (NC, 1))
        wqkvT_d, woT_d, bias_d, ident_d = jax.device_put(
            (wqkvT, woT, bias, ident), (in_sharding,) * 4)
        _WCACHE["w"] = ((w_qkv.copy(), w_out.copy(), b_out.copy()),
                        (wqkvT_d, woT_d, bias_d, ident_d))

    in_map = {"xTc": x_stack, "wqkvTc": wqkvT_d, "woTc": woT_d,
              "biasc": bias_d, "identc": ident_d}
    donate = _DONATE_NEXT if _DONATE_NEXT is not None else make_zeros()
    out_arrs = sharded(*[in_map[n] for n in in_names], *donate)
    out = np.asarray(out_arrs[out_names.index("outc")])      # [4096, 768] bf16
    _DONATE_NEXT = out_arrs
    return out.astype(np.float32)[None]
